# revision 18
# baseline (speedup 1.0000x reference)
"""Trainium2 Bass kernel for dual-attention (DisKT-style) nn module.

Math per (batch, head), S=1024, dk=64, [k, q] layout on-chip:
    sT   = (k_h @ q_h^T) + causal(-448 fp8 fixup)       fp8 matmuls
    e1   = fp8(exp(sT/8))                               ACT, fp8 out
    r1s  = 256 * sum_k e1[k, q]                         fp8 pair-DoubleRow
    otps = (vcm/4)^T @ e1                               fp8 pair-DoubleRow
    out[q>=128] = otps * recip(r1s) + vtot/1024
    exact island q<128 (second-softmax exp matters there):
      p1m  = e1 * (256*cml[k]) * recip(r1s[0:128])
      e2x  = exp(p1m)   (==1 at masked/dead keys)
      out[0:128] = ((vcm0/4)^T @ e2x + vtot0adj/4) / 256

Key simplifications vs a literal translation of the reference:
  - second softmax denominator = 1024 + sum cml*(exp(p1)-1) is in
    [1024, 1025] since sum p1 <= 1: approximated by 1024 (<=1e-3 rel).
  - the "+1 per key" of the second softmax contributes vtot[d]*rec2 ~=
    vtot/1024: folded into a per-partition scalar add (exact vtot).
  - exp(p1) ~= 1 + p1 for q >= 128 (p1 <= ~0.1): the PV/r1 matmuls run
    directly on e1 and the 1/r1 scale folds into the output pass.
  - chunk-pair DoubleRow fp8: moving AP [128, (delta,2), (1,W)] feeds
    two 128-key chunks per pass instruction at 0.5 cyc/col.

Sharding: data-parallel over batch, B=16 -> 2 per core on 8 cores.
"""

import numpy as np
import ml_dtypes

import concourse.bass as bass
import concourse.mybir as mybir
import concourse.tile as tile
from concourse import bacc
from concourse.ap import AP
from concourse.bass_utils import run_bass_kernel_spmd

B, S, D, H = 16, 1024, 512, 8
DK = D // H           # 64
NCORES = 8
BLOC = B // NCORES    # 2 batches per core
NB = BLOC * H         # 16 blocks per core
NCH = S // 128        # 8 k-chunks of 128
F32 = mybir.dt.float32
BF16 = mybir.dt.bfloat16
F8 = mybir.dt.float8e4
DR = mybir.MatmulPerfMode.DoubleRow
NPBF16 = ml_dtypes.bfloat16
NPF8 = ml_dtypes.float8_e4m3

LIVE = [S - 128 * c for c in range(NCH)]          # live width per chunk
OFF = [sum(LIVE[:c]) for c in range(NCH)]         # packed offset per chunk
PACK = OFF[-1] + LIVE[-1]                         # 4608
E1W = PACK + 128                                  # +128 zero cols for A-only
NW = (PACK + 1023) // 1024                        # 5 exp windows
WLEN = [min(1024, PACK - 1024 * w) for w in range(NW)]

# knobs test.py can flip
TRACE = False
DEBUG_DUMP = False
DBG_BLK = 0
LAST_RESULTS = None


def _scores_segs():
    """[(w, s0, s1, c, d0, d1)]: packed-col segments per exp window, split
    at 512 psum-bank boundaries; (d0, d1) = local diag-fixup range or None"""
    segs = []
    for c in range(NCH):
        g0, g1 = OFF[c], OFF[c] + LIVE[c]
        bounds = sorted({g0, g1} | {x for x in range(0, PACK + 512, 512)
                                    if g0 < x < g1})
        for s0, s1 in zip(bounds[:-1], bounds[1:]):
            w = s0 // 1024
            dm = None
            if s0 < g0 + 128:
                dm = (s0 - g0, min(s1 - g0, 128))
            segs.append((w, s0, s1, c, dm))
    return segs


SCORE_SEGS = _scores_segs()


def _pair_pieces(include_cell0):
    """[(pair, a, b, start, stop, aonly)] for a pair-DoubleRow sweep over e1.
    Cells of 256 q-cols; pair p = chunks (2p, 2p+1) covers q >= 256p; the
    first 128 cols of a pair's own cell are A-only (B reads the zero pad).
    CELL-major: each cell's accumulation group opens and closes before the
    next cell starts -- the psum hardware allows only ONE open group per
    bank, so groups must never interleave within a bank."""
    out = []
    for cell in range(4):
        mem = []
        for p in range(cell):
            mem.append([p, 256 * cell, 256 * cell + 256, False, False, False])
        if include_cell0 or cell > 0:
            mem.append([cell, 256 * cell, 256 * cell + 128, False, True, True])
        mem.append([cell, 256 * cell + 128, 256 * cell + 256, False, True, False])
        if cell == 0:
            for m in mem:
                m[3] = True
        else:
            mem[0][3] = True
        out += [(m[0], m[1], m[2], m[3], m[4], m[5]) for m in mem]
    return out


R1_PIECES = _pair_pieces(True)     # 15 pieces, covers q in [0, 1024)
PV_PIECES = _pair_pieces(False)    # 14 pieces, covers q in [128, 1024)


def build_nc(debug=False):
    nc = bacc.Bacc("TRN2", target_bir_lowering=False, debug=debug)
    AF = mybir.ActivationFunctionType
    ALU = mybir.AluOpType

    qt_d = nc.dram_tensor("qt", [BLOC, H, DK, S], F8, kind="ExternalInput")
    kt_d = nc.dram_tensor("kt", [BLOC, H, DK, S], F8, kind="ExternalInput")
    # pair-DR PV weights: vcm/4 as [pair, key, slot, d]
    vcp_d = nc.dram_tensor("vcp", [BLOC, H, 128, 4, 2, 128], F8,
                           kind="ExternalInput")
    # exact-path chunk-0 weights (cml*v)/4
    vcm0_d = nc.dram_tensor("vcm0", [BLOC, H, 128, 128], BF16,
                            kind="ExternalInput")
    vt1024_d = nc.dram_tensor("vt1024", [128, NB], F32, kind="ExternalInput")
    vt0adj_d = nc.dram_tensor("vt0adj", [128, NB], F32, kind="ExternalInput")
    cml256_d = nc.dram_tensor("cml256", [128, BLOC], F32, kind="ExternalInput")
    ones256_d = nc.dram_tensor("ones256", [128, 2, 128], F8, kind="ExternalInput")
    id8_d = nc.dram_tensor("id8", [128, 128], F8, kind="ExternalInput")
    dm8_d = nc.dram_tensor("dm8", [128, 128], F8, kind="ExternalInput")
    out1_d = nc.dram_tensor("out1t", [BLOC, D, S], BF16, kind="ExternalOutput")
    out2_d = nc.dram_tensor("out2t", [BLOC, D, S], BF16, kind="ExternalOutput")
    if DEBUG_DUMP:
        e1dump_d = nc.dram_tensor("e1dump", [128, E1W], F8, kind="ExternalOutput")
        r1dump_d = nc.dram_tensor("r1dump", [128, 1024], F32, kind="ExternalOutput")
        otdump_d = nc.dram_tensor("otdump", [128, 1024], F32, kind="ExternalOutput")

    with tile.TileContext(nc) as tc:
        with (
            tc.tile_pool(name="consts", bufs=1) as consts,
            tc.tile_pool(name="vc", bufs=3) as vcp_p,
            tc.tile_pool(name="xs", bufs=2) as xsp,
            tc.tile_pool(name="fin", bufs=2) as finp,
            tc.tile_pool(name="outs", bufs=2) as outp,
            tc.tile_pool(name="ps", bufs=1, space="PSUM") as psp,
        ):
            id8_sb = consts.tile([128, 128], F8)
            nc.sync.dma_start(out=id8_sb, in_=id8_d[:, :])
            dm8_sb = consts.tile([128, 128], F8)
            nc.sync.dma_start(out=dm8_sb, in_=dm8_d[:, :])
            ones256_sb = consts.tile([128, 2, 128], F8)
            nc.sync.dma_start(out=ones256_sb, in_=ones256_d[:])
            vt1024_sb = consts.tile([128, NB], F32)
            nc.sync.dma_start(out=vt1024_sb, in_=vt1024_d[:, :])
            vt0adj_sb = consts.tile([128, NB], F32)
            nc.sync.dma_start(out=vt0adj_sb, in_=vt0adj_d[:, :])
            cml256_sb = consts.tile([128, BLOC], F32)
            nc.sync.dma_start(out=cml256_sb, in_=cml256_d[:, :])
            negtwo_sb = consts.tile([128, 1], F32, name="negtwo")
            nc.vector.memset(negtwo_sb, -3.5)

            # persistent 3-deep rings for qt/kt/e1 (stable identity so the
            # one-time pad memsets cover all blocks)
            qt_t = [consts.tile([128, S], F8, name=f"qtr{r}") for r in range(4)]
            kt_t = [consts.tile([128, S], F8, name=f"ktr{r}") for r in range(4)]
            e1_t = [consts.tile([128, E1W], F8, name=f"e1r{r}") for r in range(3)]
            for r in range(4):
                nc.vector.memset(qt_t[r][DK:128, :], 0.0)
                nc.vector.memset(kt_t[r][DK:128, :], 0.0)
            for r in range(3):
                nc.vector.memset(e1_t[r][:, PACK:E1W], 0.0)

            # persistent psum: scores ring (2x 1024), otps, r1s
            ring = psp.tile([128, 2048], F32, name="ring")
            otps = psp.tile([128, 1024], F32, name="otps")
            r1s = psp.tile([128, 1024], F32, name="r1s")

            st = [dict() for _ in range(NB)]

            # PE warmup (DVFS pre-ramp) on a zeroed scratch
            wu_sb = consts.tile([128, 512], BF16, name="wu_sb")
            nc.vector.memset(wu_sb, 0.0)
            for r in range(12):
                nc.tensor.matmul(
                    ring[:, 0:512], lhsT=wu_sb[:, 0:128], rhs=wu_sb,
                    start=True, stop=True, skip_group_check=True,
                )

            def dma_in(blk):
                bi, h = divmod(blk, H)
                s = st[blk]
                qt_sb = qt_t[blk % 4]
                kt_sb = kt_t[blk % 4]
                nc.sync.dma_start(out=qt_sb[0:DK, :], in_=qt_d[bi, h])
                nc.sync.dma_start(out=kt_sb[0:DK, :], in_=kt_d[bi, h])
                vcp_sb = vcp_p.tile([128, 4, 2, 128], F8, tag="vcp")
                nc.sync.dma_start(out=vcp_sb, in_=vcp_d[bi, h])
                vcm0_sb = vcp_p.tile([128, 128], BF16, tag="vcm0")
                nc.sync.dma_start(out=vcm0_sb, in_=vcm0_d[bi, h])
                s["qt"], s["kt"], s["vcp"], s["vcm0"] = qt_sb, kt_sb, vcp_sb, vcm0_sb

            def alloc_e1(blk):
                st[blk]["e1"] = e1_t[blk % 3]

            def emit_scores(blk, w):
                s = st[blk]
                base = (w % 2) * 1024
                g0 = 1024 * w
                for (sw, s0, s1, c, dm) in SCORE_SEGS:
                    if sw != w:
                        continue
                    loc = base + s0 - g0
                    qa = s0 - OFF[c] + 128 * c
                    nc.tensor.matmul(
                        ring[:, loc:loc + (s1 - s0)],
                        lhsT=s["kt"][:, 128 * c:128 * c + 128],
                        rhs=s["qt"][:, qa:qa + (s1 - s0)],
                        start=True, stop=dm is None, skip_group_check=True,
                    )
                    if dm is not None:
                        d0, d1 = dm
                        nc.tensor.matmul(
                            ring[:, loc:loc + (d1 - d0)],
                            lhsT=id8_sb, rhs=dm8_sb[:, d0:d1],
                            start=False, stop=True, skip_group_check=True,
                        )

            def emit_exp1(blk, w):
                s = st[blk]
                base = (w % 2) * 1024
                # bias -3.5: e1 scaled by e^-3.5 so fp8 max 240 is never hit
                # (global max score ~67.8 -> e1max ~144)
                # (every consumer is scale-invariant in e1)
                nc.scalar.activation(
                    s["e1"][:, 1024 * w:1024 * w + WLEN[w]],
                    ring[:, base:base + WLEN[w]],
                    AF.Exp, scale=0.125, bias=negtwo_sb[:, 0:1],
                )

            def pair_mm(s, psum, pieces_sel, lhsT_of):
                """emit pair-DR matmuls for the given piece list"""
                e1t = s["e1"]
                for (p, a, b, st_, sp_, aonly) in pieces_sel:
                    a0 = OFF[2 * p] + (a - 256 * p)
                    delta = (PACK - a0) if aonly else (LIVE[2 * p] - 128)
                    rhs = AP(e1t[:, 0:1].tensor, a0,
                             [[E1W, 128], [delta, 2], [1, b - a]])
                    nc.tensor.matmul(
                        psum[:, a:b], lhsT=lhsT_of(p), rhs=rhs,
                        start=st_, stop=sp_, perf_mode=DR,
                        skip_group_check=True,
                    )

            def quanta(blk):
                """PE/DVE/DMA quanta for block blk's passes + finalize, plus
                the exact-path tail of block blk-1.  Ordering rule: a psum
                READ (DVE) is never emitted right after its producing matmul
                -- several quanta of unrelated PE work sit in between so the
                PE psum-write pipeline has drained by the time the read's
                semaphore fires (observed transient-garbage reads on HW
                otherwise)."""
                bi, h = divmod(blk, H)
                s = st[blk]
                qs = []

                # r1 pass piece for cell 0 first (unlocks exact path)
                qs.append(lambda: pair_mm(s, r1s, R1_PIECES[0:1],
                                          lambda p: ones256_sb[:, :, :]))

                rest = R1_PIECES[1:]
                for k in range(0, len(rest), 2):
                    chunk = rest[k:k + 2]
                    qs.append(lambda ch=chunk: pair_mm(
                        s, r1s, ch, lambda p: ones256_sb[:, :, :]))

                def q_exact_head():
                    rec1x = xsp.tile([128, 128], F32, tag="rec1x")
                    nc.vector.reciprocal_approx_fast(out=rec1x, in_=r1s[:, 0:128])
                    p1m = xsp.tile([128, 128], F32, tag="p1m")
                    nc.vector.scalar_tensor_tensor(
                        out=p1m, in0=s["e1"][:, 0:128],
                        scalar=cml256_sb[:, bi:bi + 1], in1=rec1x,
                        op0=ALU.mult, op1=ALU.mult,
                    )
                    s["p1m"] = p1m

                qs.append(q_exact_head)

                for k in range(0, len(PV_PIECES), 2):
                    chunk = PV_PIECES[k:k + 2]
                    qs.append(lambda ch=chunk: pair_mm(
                        s, otps, ch, lambda p: s["vcp"][:, p, :, :]))

                # exact-path matmul of the PREVIOUS block (its e2x landed at
                # the end of the previous ACT slot); doubles as PE spacing
                # between the last PV piece and the otps reads below
                if blk >= 1:
                    qs.append(lambda: exact_mm(blk - 1))

                def q_grec():
                    if DEBUG_DUMP and blk == DBG_BLK:
                        r1c = finp.tile([128, 1024], F32, tag="r1c")
                        nc.vector.tensor_copy(out=r1c, in_=r1s[:, 0:1024])
                        nc.sync.dma_start(out=r1dump_d[:, :], in_=r1c)
                        nc.sync.dma_start(out=e1dump_d[:, :], in_=s["e1"][:, :])
                    grec = finp.tile([128, 896], F32, tag="grec")
                    nc.vector.reciprocal_approx_fast(out=grec, in_=r1s[:, 128:1024])
                    s["grec"] = grec

                qs.append(q_grec)

                def q_fin_main():
                    if DEBUG_DUMP and blk == DBG_BLK:
                        otc = finp.tile([128, 1024], F32, tag="otc")
                        nc.vector.tensor_copy(out=otc, in_=otps[:, 0:1024])
                        nc.sync.dma_start(out=otdump_d[:, :], in_=otc)
                    t_sb = finp.tile([128, 896], BF16, tag="t")
                    nc.vector.tensor_mul(t_sb, otps[:, 128:1024], s["grec"])
                    out_sb = outp.tile([128, 1024], BF16, tag="out")
                    s["out"] = out_sb
                    nc.vector.tensor_scalar_add(
                        out_sb[:, 128:1024], t_sb, vt1024_sb[:, blk:blk + 1]
                    )

                qs.append(q_fin_main)

                # exact finalize of the previous block: the DVE is several
                # ops past PVex by now
                if blk >= 1:
                    qs.append(lambda: exact_fin(blk - 1))
                return qs

            def emit_e2x(blk):
                s = st[blk]
                e2x = xsp.tile([128, 128], BF16, tag="e2x")
                nc.scalar.activation(e2x, s["p1m"], AF.Exp)
                s["e2x"] = e2x

            def exact_mm(blk):
                s = st[blk]
                nc.tensor.matmul(
                    otps[:, 0:128], lhsT=s["vcm0"], rhs=s["e2x"],
                    start=True, stop=True, skip_group_check=True,
                )

            def exact_fin(blk):
                bi, h = divmod(blk, H)
                s = st[blk]
                nc.vector.tensor_scalar(
                    out=s["out"][:, 0:128], in0=otps[:, 0:128],
                    scalar1=vt0adj_sb[:, blk:blk + 1], scalar2=1.0 / 256.0,
                    op0=ALU.add, op1=ALU.mult,
                )
                nc.sync.dma_start(
                    out=out1_d[bi, DK * h:DK * (h + 1), :], in_=s["out"][0:DK, :])
                nc.sync.dma_start(
                    out=out2_d[bi, DK * h:DK * (h + 1), :], in_=s["out"][DK:2 * DK, :])

            # ---- main pipeline ----
            dma_in(0)
            dma_in(1)
            for i in range(NB + 1):
                dq = quanta(i - 1) if i >= 1 else []
                di = 0

                def drain(k):
                    nonlocal di
                    n = min(k, len(dq) - di)
                    for _ in range(n):
                        dq[di]()
                        di += 1

                if i < NB:
                    if i + 2 < NB:
                        dma_in(i + 2)
                    alloc_e1(i)
                    # exp1 lags scores by one window: the psum write pipeline
                    # of window w's matmuls drains while window w+1 is issued
                    for w in range(NW):
                        emit_scores(i, w)
                        if w >= 1:
                            emit_exp1(i, w - 1)
                        drain(1 if w < 2 else 2)
                    emit_exp1(i, NW - 1)
                    drain(len(dq))
                    if i >= 1:
                        emit_e2x(i - 1)
                else:
                    drain(len(dq))
                    emit_e2x(i - 1)
                    exact_mm(i - 1)
                    exact_fin(i - 1)

    nc.compile()
    return nc


_NC_CACHE = None


def _get_nc():
    global _NC_CACHE
    if _NC_CACHE is None:
        _NC_CACHE = build_nc()
    return _NC_CACHE


def make_in_maps(q, k, v1, v2, cm):
    """Full inputs -> per-core input maps (host-side sharding + layout)."""
    q = np.asarray(q, dtype=np.float32)
    k = np.asarray(k, dtype=np.float32)
    v1 = np.asarray(v1, dtype=np.float32)
    v2 = np.asarray(v2, dtype=np.float32)
    cm = np.asarray(cm)

    id8 = np.eye(128, dtype=NPF8)
    # additive causal mask on the diag block: -448 where k >= q else 0
    dm8 = np.where(
        np.arange(128)[:, None] >= np.arange(128)[None, :], -240.0, 0.0
    ).astype(NPF8)
    ones256 = np.full((128, 2, 128), 128.0, NPF8)

    in_maps = []
    for core in range(NCORES):
        b0 = core * BLOC
        qt = np.ascontiguousarray(
            q[b0:b0 + BLOC].reshape(BLOC, S, H, DK).transpose(0, 2, 3, 1)
        ).astype(NPF8)
        kt = np.ascontiguousarray(
            k[b0:b0 + BLOC].reshape(BLOC, S, H, DK).transpose(0, 2, 3, 1)
        ).astype(NPF8)
        cml = 1.0 - cm[b0:b0 + BLOC].astype(np.float32)      # [BLOC, S]
        # v concat: [BLOC, H, key(S), d(128)]
        v1s = v1[b0:b0 + BLOC].reshape(BLOC, S, H, DK).transpose(0, 2, 1, 3)
        v2s = v2[b0:b0 + BLOC].reshape(BLOC, S, H, DK).transpose(0, 2, 1, 3)
        vc = np.concatenate([v1s, v2s], axis=3)               # [BLOC,H,S,128]
        vtot = vc.astype(np.float64).sum(axis=2).astype(np.float32)  # [BLOC,H,128]
        vcm8 = vc * (cml[:, None, :, None] * 0.125)           # masked /8
        # pair weights: [BLOC, H, pair, key128, slot, d]
        vcp = np.ascontiguousarray(
            vcm8.reshape(BLOC, H, NCH, 128, 128)
                .reshape(BLOC, H, 4, 2, 128, 128)
                .transpose(0, 1, 4, 2, 3, 5)
        ).astype(NPF8)
        vcm0 = np.ascontiguousarray(2.0 * vcm8[:, :, 0:128, :])
        vcm0 = vcm0.astype(NPBF16)                            # [BLOC,H,128,128]
        vt1024 = np.ascontiguousarray(
            (vtot / 1024.0).reshape(NB, 128).T.astype(np.float32))
        vt0adj = np.ascontiguousarray(
            ((vtot - (vc[:, :, 0:128, :] * cml[:, None, 0:128, None]).sum(2))
             / 4.0).reshape(NB, 128).T.astype(np.float32))
        cml256 = np.ascontiguousarray((128.0 * cml[:, 0:128]).T.astype(np.float32))
        in_maps.append(
            dict(qt=qt, kt=kt, vcp=vcp, vcm0=vcm0, vt1024=vt1024,
                 vt0adj=vt0adj, cml256=cml256, ones256=ones256,
                 id8=id8, dm8=dm8)
        )
    return in_maps


def _gather(res):
    out1 = np.concatenate(
        [np.asarray(r["out1t"]).astype(np.float32).transpose(0, 2, 1)
         for r in res.results], axis=0)
    out2 = np.concatenate(
        [np.asarray(r["out2t"]).astype(np.float32).transpose(0, 2, 1)
         for r in res.results], axis=0)
    out1[:, 0, :] = 0.0
    out2[:, 0, :] = 0.0
    return np.ascontiguousarray(out1), np.ascontiguousarray(out2)


def kernel(q, k, v1, v2, counter_attention_mask):
    global LAST_RESULTS
    in_maps = make_in_maps(q, k, v1, v2, counter_attention_mask)
    nc = _get_nc()
    res = run_bass_kernel_spmd(
        nc, in_maps, core_ids=list(range(NCORES)), trace=TRACE
    )
    LAST_RESULTS = res
    return _gather(res)


# revision 19
# speedup vs baseline: 1.0008x; 1.0008x over previous
"""Trainium2 Bass kernel for dual-attention (DisKT-style) nn module.

Math per (batch, head), S=1024, dk=64, [k, q] layout on-chip:
    sT   = (k_h @ q_h^T) + causal(-448 fp8 fixup)       fp8 matmuls
    e1   = fp8(exp(sT/8))                               ACT, fp8 out
    r1s  = 256 * sum_k e1[k, q]                         fp8 pair-DoubleRow
    otps = (vcm/4)^T @ e1                               fp8 pair-DoubleRow
    out[q>=128] = otps * recip(r1s) + vtot/1024
    exact island q<128 (second-softmax exp matters there):
      p1m  = e1 * (256*cml[k]) * recip(r1s[0:128])
      e2x  = exp(p1m)   (==1 at masked/dead keys)
      out[0:128] = ((vcm0/4)^T @ e2x + vtot0adj/4) / 256

Key simplifications vs a literal translation of the reference:
  - second softmax denominator = 1024 + sum cml*(exp(p1)-1) is in
    [1024, 1025] since sum p1 <= 1: approximated by 1024 (<=1e-3 rel).
  - the "+1 per key" of the second softmax contributes vtot[d]*rec2 ~=
    vtot/1024: folded into a per-partition scalar add (exact vtot).
  - exp(p1) ~= 1 + p1 for q >= 128 (p1 <= ~0.1): the PV/r1 matmuls run
    directly on e1 and the 1/r1 scale folds into the output pass.
  - chunk-pair DoubleRow fp8: moving AP [128, (delta,2), (1,W)] feeds
    two 128-key chunks per pass instruction at 0.5 cyc/col.

Sharding: data-parallel over batch, B=16 -> 2 per core on 8 cores.
"""

import numpy as np
import ml_dtypes

import concourse.bass as bass
import concourse.mybir as mybir
import concourse.tile as tile
from concourse import bacc
from concourse.ap import AP
from concourse.bass_utils import run_bass_kernel_spmd

B, S, D, H = 16, 1024, 512, 8
DK = D // H           # 64
NCORES = 8
BLOC = B // NCORES    # 2 batches per core
NB = BLOC * H         # 16 blocks per core
NCH = S // 128        # 8 k-chunks of 128
F32 = mybir.dt.float32
BF16 = mybir.dt.bfloat16
F8 = mybir.dt.float8e4
DR = mybir.MatmulPerfMode.DoubleRow
NPBF16 = ml_dtypes.bfloat16
NPF8 = ml_dtypes.float8_e4m3

LIVE = [S - 128 * c for c in range(NCH)]          # live width per chunk
OFF = [sum(LIVE[:c]) for c in range(NCH)]         # packed offset per chunk
PACK = OFF[-1] + LIVE[-1]                         # 4608
E1W = PACK + 128                                  # +128 zero cols for A-only
NW = (PACK + 1023) // 1024                        # 5 exp windows
WLEN = [min(1024, PACK - 1024 * w) for w in range(NW)]

# knobs test.py can flip
TRACE = False
DEBUG_DUMP = False
DBG_BLK = 0
LAST_RESULTS = None


def _scores_segs():
    """[(w, s0, s1, c, d0, d1)]: packed-col segments per exp window, split
    at 512 psum-bank boundaries; (d0, d1) = local diag-fixup range or None"""
    segs = []
    for c in range(NCH):
        g0, g1 = OFF[c], OFF[c] + LIVE[c]
        bounds = sorted({g0, g1} | {x for x in range(0, PACK + 512, 512)
                                    if g0 < x < g1})
        for s0, s1 in zip(bounds[:-1], bounds[1:]):
            w = s0 // 1024
            dm = None
            if s0 < g0 + 128:
                dm = (s0 - g0, min(s1 - g0, 128))
            segs.append((w, s0, s1, c, dm))
    return segs


SCORE_SEGS = _scores_segs()


def _pair_pieces(include_cell0):
    """[(pair, a, b, start, stop, aonly)] for a pair-DoubleRow sweep over e1.
    Cells of 256 q-cols; pair p = chunks (2p, 2p+1) covers q >= 256p; the
    first 128 cols of a pair's own cell are A-only (B reads the zero pad).
    CELL-major: each cell's accumulation group opens and closes before the
    next cell starts -- the psum hardware allows only ONE open group per
    bank, so groups must never interleave within a bank."""
    out = []
    for cell in range(4):
        mem = []
        for p in range(cell):
            mem.append([p, 256 * cell, 256 * cell + 256, False, False, False])
        if include_cell0 or cell > 0:
            mem.append([cell, 256 * cell, 256 * cell + 128, False, True, True])
        mem.append([cell, 256 * cell + 128, 256 * cell + 256, False, True, False])
        if cell == 0:
            for m in mem:
                m[3] = True
        else:
            mem[0][3] = True
        out += [(m[0], m[1], m[2], m[3], m[4], m[5]) for m in mem]
    return out


R1_PIECES = _pair_pieces(True)     # 15 pieces, covers q in [0, 1024)
PV_PIECES = _pair_pieces(False)    # 14 pieces, covers q in [128, 1024)


def build_nc(debug=False):
    nc = bacc.Bacc("TRN2", target_bir_lowering=False, debug=debug)
    AF = mybir.ActivationFunctionType
    ALU = mybir.AluOpType

    qt_d = nc.dram_tensor("qt", [BLOC, H, DK, S], F8, kind="ExternalInput")
    kt_d = nc.dram_tensor("kt", [BLOC, H, DK, S], F8, kind="ExternalInput")
    # pair-DR PV weights: vcm/4 as [pair, key, slot, d]
    vcp_d = nc.dram_tensor("vcp", [BLOC, H, 128, 4, 2, 128], F8,
                           kind="ExternalInput")
    # exact-path chunk-0 weights (cml*v)/4
    vcm0_d = nc.dram_tensor("vcm0", [BLOC, H, 128, 128], BF16,
                            kind="ExternalInput")
    vt1024_d = nc.dram_tensor("vt1024", [128, NB], F32, kind="ExternalInput")
    vt0adj_d = nc.dram_tensor("vt0adj", [128, NB], F32, kind="ExternalInput")
    cml256_d = nc.dram_tensor("cml256", [128, BLOC], F32, kind="ExternalInput")
    ones256_d = nc.dram_tensor("ones256", [128, 2, 128], F8, kind="ExternalInput")
    id8_d = nc.dram_tensor("id8", [128, 128], F8, kind="ExternalInput")
    dm8_d = nc.dram_tensor("dm8", [128, 128], F8, kind="ExternalInput")
    out1_d = nc.dram_tensor("out1t", [BLOC, D, S], BF16, kind="ExternalOutput")
    out2_d = nc.dram_tensor("out2t", [BLOC, D, S], BF16, kind="ExternalOutput")
    if DEBUG_DUMP:
        e1dump_d = nc.dram_tensor("e1dump", [128, E1W], F8, kind="ExternalOutput")
        r1dump_d = nc.dram_tensor("r1dump", [128, 1024], F32, kind="ExternalOutput")
        otdump_d = nc.dram_tensor("otdump", [128, 1024], F32, kind="ExternalOutput")

    with tile.TileContext(nc) as tc:
        with (
            tc.tile_pool(name="consts", bufs=1) as consts,
            tc.tile_pool(name="vc", bufs=3) as vcp_p,
            tc.tile_pool(name="xs", bufs=2) as xsp,
            tc.tile_pool(name="fin", bufs=2) as finp,
            tc.tile_pool(name="outs", bufs=2) as outp,
            tc.tile_pool(name="ps", bufs=1, space="PSUM") as psp,
        ):
            id8_sb = consts.tile([128, 128], F8)
            nc.sync.dma_start(out=id8_sb, in_=id8_d[:, :])
            dm8_sb = consts.tile([128, 128], F8)
            nc.sync.dma_start(out=dm8_sb, in_=dm8_d[:, :])
            ones256_sb = consts.tile([128, 2, 128], F8)
            nc.sync.dma_start(out=ones256_sb, in_=ones256_d[:])
            vt1024_sb = consts.tile([128, NB], F32)
            nc.sync.dma_start(out=vt1024_sb, in_=vt1024_d[:, :])
            vt0adj_sb = consts.tile([128, NB], F32)
            nc.sync.dma_start(out=vt0adj_sb, in_=vt0adj_d[:, :])
            cml256_sb = consts.tile([128, BLOC], F32)
            nc.sync.dma_start(out=cml256_sb, in_=cml256_d[:, :])
            negtwo_sb = consts.tile([128, 1], F32, name="negtwo")
            nc.vector.memset(negtwo_sb, -3.5)

            # persistent 3-deep rings for qt/kt/e1 (stable identity so the
            # one-time pad memsets cover all blocks)
            qt_t = [consts.tile([128, S], F8, name=f"qtr{r}") for r in range(4)]
            kt_t = [consts.tile([128, S], F8, name=f"ktr{r}") for r in range(4)]
            e1_t = [consts.tile([128, E1W], F8, name=f"e1r{r}") for r in range(3)]
            for r in range(4):
                nc.vector.memset(qt_t[r][DK:128, :], 0.0)
                nc.vector.memset(kt_t[r][DK:128, :], 0.0)
            for r in range(3):
                nc.vector.memset(e1_t[r][:, PACK:E1W], 0.0)

            # persistent psum: scores ring (2x 1024), otps, r1s
            ring = psp.tile([128, 2048], F32, name="ring")
            otps = psp.tile([128, 1024], F32, name="otps")
            r1s = psp.tile([128, 1024], F32, name="r1s")

            st = [dict() for _ in range(NB)]

            # PE warmup (DVFS pre-ramp) on a zeroed scratch
            wu_sb = consts.tile([128, 512], BF16, name="wu_sb")
            nc.vector.memset(wu_sb, 0.0)
            for r in range(12):
                nc.tensor.matmul(
                    ring[:, 0:512], lhsT=wu_sb[:, 0:128], rhs=wu_sb,
                    start=True, stop=True, skip_group_check=True,
                )

            def dma_in(blk):
                bi, h = divmod(blk, H)
                s = st[blk]
                qt_sb = qt_t[blk % 4]
                kt_sb = kt_t[blk % 4]
                nc.sync.dma_start(out=qt_sb[0:DK, :], in_=qt_d[bi, h])
                nc.sync.dma_start(out=kt_sb[0:DK, :], in_=kt_d[bi, h])
                vcp_sb = vcp_p.tile([128, 4, 2, 128], F8, tag="vcp")
                nc.sync.dma_start(out=vcp_sb, in_=vcp_d[bi, h])
                vcm0_sb = vcp_p.tile([128, 128], BF16, tag="vcm0")
                nc.sync.dma_start(out=vcm0_sb, in_=vcm0_d[bi, h])
                s["qt"], s["kt"], s["vcp"], s["vcm0"] = qt_sb, kt_sb, vcp_sb, vcm0_sb

            def alloc_e1(blk):
                st[blk]["e1"] = e1_t[blk % 3]

            def emit_scores(blk, w):
                s = st[blk]
                base = (w % 2) * 1024
                g0 = 1024 * w
                for (sw, s0, s1, c, dm) in SCORE_SEGS:
                    if sw != w:
                        continue
                    loc = base + s0 - g0
                    qa = s0 - OFF[c] + 128 * c
                    nc.tensor.matmul(
                        ring[:, loc:loc + (s1 - s0)],
                        lhsT=s["kt"][:, 128 * c:128 * c + 128],
                        rhs=s["qt"][:, qa:qa + (s1 - s0)],
                        start=True, stop=dm is None, skip_group_check=True,
                    )
                    if dm is not None:
                        d0, d1 = dm
                        nc.tensor.matmul(
                            ring[:, loc:loc + (d1 - d0)],
                            lhsT=id8_sb, rhs=dm8_sb[:, d0:d1],
                            start=False, stop=True, skip_group_check=True,
                        )

            def emit_exp1(blk, w):
                s = st[blk]
                base = (w % 2) * 1024
                # bias -3.5: e1 scaled by e^-3.5 so fp8 max 240 is never hit
                # (global max score ~67.8 -> e1max ~144)
                # (every consumer is scale-invariant in e1)
                nc.scalar.activation(
                    s["e1"][:, 1024 * w:1024 * w + WLEN[w]],
                    ring[:, base:base + WLEN[w]],
                    AF.Exp, scale=0.125, bias=negtwo_sb[:, 0:1],
                )

            def pair_mm(s, psum, pieces_sel, lhsT_of):
                """emit pair-DR matmuls for the given piece list"""
                e1t = s["e1"]
                for (p, a, b, st_, sp_, aonly) in pieces_sel:
                    a0 = OFF[2 * p] + (a - 256 * p)
                    delta = (PACK - a0) if aonly else (LIVE[2 * p] - 128)
                    rhs = AP(e1t[:, 0:1].tensor, a0,
                             [[E1W, 128], [delta, 2], [1, b - a]])
                    nc.tensor.matmul(
                        psum[:, a:b], lhsT=lhsT_of(p), rhs=rhs,
                        start=st_, stop=sp_, perf_mode=DR,
                        skip_group_check=True,
                    )

            def quanta(blk):
                """PE/DVE/DMA quanta for block blk's passes + finalize, plus
                the exact-path tail of block blk-1.  Ordering rule: a psum
                READ (DVE) is never emitted right after its producing matmul
                -- several quanta of unrelated PE work sit in between so the
                PE psum-write pipeline has drained by the time the read's
                semaphore fires (observed transient-garbage reads on HW
                otherwise)."""
                bi, h = divmod(blk, H)
                s = st[blk]
                qs = []

                # r1 pass piece for cell 0 first (unlocks exact path)
                qs.append(lambda: pair_mm(s, r1s, R1_PIECES[0:1],
                                          lambda p: ones256_sb[:, :, :]))

                rest = R1_PIECES[1:]
                for k in range(0, len(rest), 2):
                    chunk = rest[k:k + 2]
                    qs.append(lambda ch=chunk: pair_mm(
                        s, r1s, ch, lambda p: ones256_sb[:, :, :]))

                def q_exact_head():
                    rec1x = xsp.tile([128, 128], F32, tag="rec1x")
                    nc.vector.reciprocal_approx_fast(out=rec1x, in_=r1s[:, 0:128])
                    p1m = xsp.tile([128, 128], F32, tag="p1m")
                    nc.vector.scalar_tensor_tensor(
                        out=p1m, in0=s["e1"][:, 0:128],
                        scalar=cml256_sb[:, bi:bi + 1], in1=rec1x,
                        op0=ALU.mult, op1=ALU.mult,
                    )
                    s["p1m"] = p1m

                qs.append(q_exact_head)

                for k in range(0, len(PV_PIECES), 2):
                    chunk = PV_PIECES[k:k + 2]
                    qs.append(lambda ch=chunk: pair_mm(
                        s, otps, ch, lambda p: s["vcp"][:, p, :, :]))

                # exact-path matmul of the PREVIOUS block (its e2x landed at
                # the end of the previous ACT slot); doubles as PE spacing
                # between the last PV piece and the otps reads below
                if blk >= 1:
                    qs.append(lambda: exact_mm(blk - 1))

                def q_grec():
                    if DEBUG_DUMP and blk == DBG_BLK:
                        r1c = finp.tile([128, 1024], F32, tag="r1c")
                        nc.vector.tensor_copy(out=r1c, in_=r1s[:, 0:1024])
                        nc.sync.dma_start(out=r1dump_d[:, :], in_=r1c)
                        nc.sync.dma_start(out=e1dump_d[:, :], in_=s["e1"][:, :])
                    grec = finp.tile([128, 896], F32, tag="grec")
                    nc.vector.reciprocal_approx_fast(out=grec, in_=r1s[:, 128:1024])
                    s["grec"] = grec

                qs.append(q_grec)

                def q_fin_main():
                    if DEBUG_DUMP and blk == DBG_BLK:
                        otc = finp.tile([128, 1024], F32, tag="otc")
                        nc.vector.tensor_copy(out=otc, in_=otps[:, 0:1024])
                        nc.sync.dma_start(out=otdump_d[:, :], in_=otc)
                    t_sb = finp.tile([128, 896], BF16, tag="t")
                    nc.vector.tensor_mul(t_sb, otps[:, 128:1024], s["grec"])
                    out_sb = outp.tile([128, 1024], BF16, tag="out")
                    s["out"] = out_sb
                    nc.vector.tensor_scalar_add(
                        out_sb[:, 128:1024], t_sb, vt1024_sb[:, blk:blk + 1]
                    )

                qs.append(q_fin_main)

                # exact finalize of the previous block: the DVE is several
                # ops past PVex by now
                if blk >= 1:
                    qs.append(lambda: exact_fin(blk - 1))
                return qs

            def emit_e2x(blk):
                s = st[blk]
                e2x = xsp.tile([128, 128], BF16, tag="e2x")
                nc.scalar.activation(e2x, s["p1m"], AF.Exp)
                s["e2x"] = e2x

            def exact_mm(blk):
                s = st[blk]
                nc.tensor.matmul(
                    otps[:, 0:128], lhsT=s["vcm0"], rhs=s["e2x"],
                    start=True, stop=True, skip_group_check=True,
                )

            def exact_fin(blk):
                bi, h = divmod(blk, H)
                s = st[blk]
                nc.vector.tensor_scalar(
                    out=s["out"][:, 0:128], in0=otps[:, 0:128],
                    scalar1=vt0adj_sb[:, blk:blk + 1], scalar2=1.0 / 256.0,
                    op0=ALU.add, op1=ALU.mult,
                )
                nc.sync.dma_start(
                    out=out1_d[bi, DK * h:DK * (h + 1), :], in_=s["out"][0:DK, :])
                nc.sync.dma_start(
                    out=out2_d[bi, DK * h:DK * (h + 1), :], in_=s["out"][DK:2 * DK, :])

            # ---- main pipeline ----
            dma_in(0)
            dma_in(1)
            for i in range(NB + 1):
                dq = quanta(i - 1) if i >= 1 else []
                di = 0

                def drain(k):
                    nonlocal di
                    n = min(k, len(dq) - di)
                    for _ in range(n):
                        dq[di]()
                        di += 1

                if i < NB:
                    if i + 2 < NB:
                        dma_in(i + 2)
                    alloc_e1(i)
                    # exp1 lags scores by one window: the psum write pipeline
                    # of window w's matmuls drains while window w+1 is issued
                    for w in range(NW):
                        emit_scores(i, w)
                        if w >= 1:
                            emit_exp1(i, w - 1)
                        drain(0 if w == 0 else 3)
                    emit_exp1(i, NW - 1)
                    drain(len(dq))
                    if i >= 1:
                        emit_e2x(i - 1)
                else:
                    drain(len(dq))
                    emit_e2x(i - 1)
                    exact_mm(i - 1)
                    exact_fin(i - 1)

    nc.compile()
    return nc


_NC_CACHE = None


def _get_nc():
    global _NC_CACHE
    if _NC_CACHE is None:
        _NC_CACHE = build_nc()
    return _NC_CACHE


def make_in_maps(q, k, v1, v2, cm):
    """Full inputs -> per-core input maps (host-side sharding + layout)."""
    q = np.asarray(q, dtype=np.float32)
    k = np.asarray(k, dtype=np.float32)
    v1 = np.asarray(v1, dtype=np.float32)
    v2 = np.asarray(v2, dtype=np.float32)
    cm = np.asarray(cm)

    id8 = np.eye(128, dtype=NPF8)
    # additive causal mask on the diag block: -448 where k >= q else 0
    dm8 = np.where(
        np.arange(128)[:, None] >= np.arange(128)[None, :], -240.0, 0.0
    ).astype(NPF8)
    ones256 = np.full((128, 2, 128), 128.0, NPF8)

    in_maps = []
    for core in range(NCORES):
        b0 = core * BLOC
        qt = np.ascontiguousarray(
            q[b0:b0 + BLOC].reshape(BLOC, S, H, DK).transpose(0, 2, 3, 1)
        ).astype(NPF8)
        kt = np.ascontiguousarray(
            k[b0:b0 + BLOC].reshape(BLOC, S, H, DK).transpose(0, 2, 3, 1)
        ).astype(NPF8)
        cml = 1.0 - cm[b0:b0 + BLOC].astype(np.float32)      # [BLOC, S]
        # v concat: [BLOC, H, key(S), d(128)]
        v1s = v1[b0:b0 + BLOC].reshape(BLOC, S, H, DK).transpose(0, 2, 1, 3)
        v2s = v2[b0:b0 + BLOC].reshape(BLOC, S, H, DK).transpose(0, 2, 1, 3)
        vc = np.concatenate([v1s, v2s], axis=3)               # [BLOC,H,S,128]
        vtot = vc.astype(np.float64).sum(axis=2).astype(np.float32)  # [BLOC,H,128]
        vcm8 = vc * (cml[:, None, :, None] * 0.125)           # masked /8
        # pair weights: [BLOC, H, pair, key128, slot, d]
        vcp = np.ascontiguousarray(
            vcm8.reshape(BLOC, H, NCH, 128, 128)
                .reshape(BLOC, H, 4, 2, 128, 128)
                .transpose(0, 1, 4, 2, 3, 5)
        ).astype(NPF8)
        vcm0 = np.ascontiguousarray(2.0 * vcm8[:, :, 0:128, :])
        vcm0 = vcm0.astype(NPBF16)                            # [BLOC,H,128,128]
        vt1024 = np.ascontiguousarray(
            (vtot / 1024.0).reshape(NB, 128).T.astype(np.float32))
        vt0adj = np.ascontiguousarray(
            ((vtot - (vc[:, :, 0:128, :] * cml[:, None, 0:128, None]).sum(2))
             / 4.0).reshape(NB, 128).T.astype(np.float32))
        cml256 = np.ascontiguousarray((128.0 * cml[:, 0:128]).T.astype(np.float32))
        in_maps.append(
            dict(qt=qt, kt=kt, vcp=vcp, vcm0=vcm0, vt1024=vt1024,
                 vt0adj=vt0adj, cml256=cml256, ones256=ones256,
                 id8=id8, dm8=dm8)
        )
    return in_maps


def _gather(res):
    out1 = np.concatenate(
        [np.asarray(r["out1t"]).astype(np.float32).transpose(0, 2, 1)
         for r in res.results], axis=0)
    out2 = np.concatenate(
        [np.asarray(r["out2t"]).astype(np.float32).transpose(0, 2, 1)
         for r in res.results], axis=0)
    out1[:, 0, :] = 0.0
    out2[:, 0, :] = 0.0
    return np.ascontiguousarray(out1), np.ascontiguousarray(out2)


def kernel(q, k, v1, v2, counter_attention_mask):
    global LAST_RESULTS
    in_maps = make_in_maps(q, k, v1, v2, counter_attention_mask)
    nc = _get_nc()
    res = run_bass_kernel_spmd(
        nc, in_maps, core_ids=list(range(NCORES)), trace=TRACE
    )
    LAST_RESULTS = res
    return _gather(res)
